# revision 4
# baseline (speedup 1.0000x reference)
"""ChaosSSMCore (diag mode) Trainium2 kernel.

Problem: B=4, S=4096, D=1024, fp32.
    delta  = softplus(x @ Wd.T); decay = exp(-delta * sigmoid(log_a))
    update = sigmoid(x @ Ws.T) * tanh(x @ Wi.T)
    gate   = sigmoid(x @ Wg.T)
    state_t = decay_t * state_{t-1} + update_t        (scan over S, elementwise in D)
    y      = (gate * states) @ Wo.T

Sharding: 8 cores = 4 batches x 2 halves of the D output dim. Each core
computes its 512-channel slice of the 4 input projections in [e, s] layout
(channels on partitions, sequence on the free axis), applies activations on
the scalar engine, runs the hardware tensor_tensor_scan (state = d*s + u
along the free dim) on the vector engine, gates, and computes a partial
output GEMM against its 512 columns of W_out. Host sums the two partials
per batch and transposes back.

All matmul operands are bf16 (host-side conversion; PSUM accumulation stays
fp32): the PE streams bf16 and fp32r at the same 1 row/cycle, but bf16
weight loads are shorter (less unhidden LDWEIGHTS time) and every input DMA
halves. Measured end-to-end rel err ~4e-3 vs the 2e-2 gate. The scan runs
in fp32; only the gated scan output (the moving operand of the output GEMM)
and the y partials written back to HBM are bf16.

DMA: aggregate bandwidth measures ~175GB/s no matter how many queues are
used, so startup is choreographed around strict priority: w0/x0 stream as
interleaved 128KB k-slices (sync/gpsimd queues), then w1,w2,w3 as k-slices,
then x(1), wo. Chunk 0's four projection blocks run K-OUTER across four
live psum banks so the PE consumes each arriving slice with 4 matmuls
instead of waiting for whole tensors, and a burst of dummy matmuls on
zeroed scratch warms the PE DVFS ramp before the first slice lands. y is
written back with one DMA per sequence chunk from a batched [P, MT*NC]
tile; psum->bf16 casts alternate Vector/Scalar so the output-GEMM psum
rotation never waits on a busy Vector engine.

Activation tables: sigmoid is computed as (1 + tanh(z/2))/2 with the 1/2
input scale folded into W_select/W_gate and the output scales into W_out
(W_out/4), so every psum-draining activation (delta-Exp + 12 Tanh) lives in
the `exp_and_others` table set. softplus(z) = ln(exp(z)+1) needs Ln from a
different set: the [4x Ln][4x decay-Exp] block (SBUF-only, 2 table loads)
is tucked into the middle of the Tanh phase where the ACT engine runs far
ahead of the PE's psum-drain needs, so the PE never waits on a table
switch. The ACT order is pinned chunk by chunk.

The last chunk's scan/gate/output GEMM run in two 256-wide halves with the
casts split across Vector and Scalar and two stores per half, so the final
HBM flush shrinks and starts earlier.
"""

import sys

if "/opt/trn_rl_repo" not in sys.path:
    sys.path.insert(0, "/opt/trn_rl_repo")

import numpy as np

# Problem constants (hardcoded per harness contract).
B, S, D = 4, 4096, 1024
P = 128           # SBUF partitions
E = D // 2        # per-core channel slice
NC = 512          # sequence chunk (= one PSUM bank of fp32)
KT = D // P       # k-tiles per input projection contraction (8)
ET = E // P       # e-tiles per core (4)
KO = E // P       # k'-tiles for the output GEMM contraction (4)
MT = D // P       # output-row tiles (8)
SC = S // NC      # sequence chunks (8)
HC = NC // 2      # half-chunk for the tail drain (256)
N_WARM = 7        # dummy warmup matmuls (PE DVFS ramp during startup DMAs)
N_CORES = 8

_CACHE = {}


def _build_program():
    import concourse.bacc as bacc
    import concourse.mybir as mybir
    import concourse.tile as tile
    from concourse.tile import add_dep_helper
    from contextlib import ExitStack

    f32 = mybir.dt.float32
    bf16 = mybir.dt.bfloat16
    AF = mybir.ActivationFunctionType
    OP = mybir.AluOpType

    nc = bacc.Bacc("TRN2", target_bir_lowering=False)

    # Host pre-arranges everything partition-major so each partition's slice
    # of any single DMA is contiguous DRAM.
    xt = nc.declare_dram_parameter("xt", [P, SC, KT, NC], bf16, isOutput=False)
    w4 = nc.declare_dram_parameter("w4", [4, P, KT, E], bf16, isOutput=False)
    wo = nc.declare_dram_parameter("wo", [P, KO * D], bf16, isOutput=False)
    na = nc.declare_dram_parameter("na", [P, ET], f32, isOutput=False)
    yt = nc.declare_dram_parameter("yt", [P, SC, MT, NC], bf16, isOutput=True)

    with tile.TileContext(nc) as tc, ExitStack() as ctx:
        wpool = ctx.enter_context(tc.tile_pool(name="w", bufs=1))
        xpool = ctx.enter_context(tc.tile_pool(name="x", bufs=2))
        ppd = ctx.enter_context(tc.tile_pool(name="ppd", bufs=3, space="PSUM"))
        pp = ctx.enter_context(tc.tile_pool(name="pp", bufs=3, space="PSUM"))
        pyp = ctx.enter_context(tc.tile_pool(name="pyp", bufs=2, space="PSUM"))
        dpool = ctx.enter_context(tc.tile_pool(name="dp", bufs=2))
        decpool = ctx.enter_context(tc.tile_pool(name="dec", bufs=8))
        spool = ctx.enter_context(tc.tile_pool(name="sp", bufs=4))
        tpool = ctx.enter_context(tc.tile_pool(name="tp", bufs=4))
        upool = ctx.enter_context(tc.tile_pool(name="up", bufs=3))
        stpool = ctx.enter_context(tc.tile_pool(name="stp", bufs=6))
        gpool = ctx.enter_context(tc.tile_pool(name="gp", bufs=4))
        gdpool = ctx.enter_context(tc.tile_pool(name="gdp", bufs=8))
        ypool = ctx.enter_context(tc.tile_pool(name="yp", bufs=2))

        # Pin the ACT instruction order to the emission order so the
        # scheduler cannot move a psum-draining activation behind a
        # table-switching phase from another chunk.
        last_act = [None]

        def act(*args, **kwargs):
            h = nc.scalar.activation(*args, **kwargs)
            if last_act[0] is not None:
                add_dep_helper(h.ins, last_act[0].ins, sync=False,
                               reason="pin ACT table phase order")
            last_act[0] = h
            return h

        # PE warmup scratch.
        xz = wpool.tile([P, NC], bf16, name="xz", tag="xz")
        wz = wpool.tile([P, P], bf16, name="wz", tag="wz")
        nc.vector.memset(xz[:, :], 0.0)
        nc.vector.memset(wz[:, :], 0.0)

        # Startup DMA choreography (shared ~175GB/s pipe, strict priority):
        #   gpsimd: na, x0 k-slices          (x0 paired with w0 stream)
        #   sync:   w0 k-slices, w1/w2/w3 k-slices, x1 k-slices, wo
        w_sb = [None] * 4
        for q in range(4):
            w_sb[q] = wpool.tile([P, KT * E], bf16, name=f"w{q}_sb", tag=f"w{q}")
        x_next = xpool.tile([P, KT * NC], bf16, name="x_sb", tag="x")
        na_sb = wpool.tile([P, ET], f32, name="na_sb", tag="na")
        wo_sb = wpool.tile([P, KO * D], bf16, name="wo_sb", tag="wo")

        nc.gpsimd.dma_start(na_sb[:, :], na[:, :])
        for k in range(KT):
            nc.sync.dma_start(w_sb[0][:, k * E:(k + 1) * E], w4[0, :, k])
            nc.gpsimd.dma_start(
                x_next[:, k * NC:(k + 1) * NC], xt[:, 0, k])
        for q in (1, 2, 3):
            for k in range(KT):
                nc.sync.dma_start(w_sb[q][:, k * E:(k + 1) * E], w4[q, :, k])
        x1 = xpool.tile([P, KT * NC], bf16, name="x_sb", tag="x")
        for k in range(KT):
            nc.sync.dma_start(x1[:, k * NC:(k + 1) * NC], xt[:, 1, k])
        nc.sync.dma_start(wo_sb[:, :], wo[:, :])

        def load_x(c):
            x_sb = xpool.tile([P, KT * NC], bf16, name="x_sb", tag="x")
            nc.gpsimd.dma_start(
                x_sb.rearrange("p (k s) -> p k s", k=KT), xt[:, c])
            return x_sb

        # Dummy warmup matmuls (zeros in, never read back).
        warm_ps = ppd.tile([P, NC], f32, name="warm", tag="ppd")
        for _ in range(N_WARM):
            nc.tensor.matmul(warm_ps[:, :], wz[:, :], xz[:, :],
                             start=True, stop=True)

        prev_states = [None] * ET
        gated_tiles = [[None] * ET for _ in range(SC)]
        front_state = {}

        def mm_group(ps, q, j, x_sb):
            for k in range(KT):
                nc.tensor.matmul(
                    ps[:, :],
                    w_sb[q][:, k * E + j * P: k * E + (j + 1) * P],
                    x_sb[:, k * NC:(k + 1) * NC],
                    start=(k == 0),
                    stop=(k == KT - 1),
                )

        def mm_block_kouter(q, banks, x_sb):
            ps = [pool.tile([P, NC], f32, name=f"ps0_{q}{j}", tag=tag)
                  for j, (pool, tag) in enumerate(banks)]
            for k in range(KT):
                for j in range(ET):
                    nc.tensor.matmul(
                        ps[j][:, :],
                        w_sb[q][:, k * E + j * P: k * E + (j + 1) * P],
                        x_sb[:, k * NC:(k + 1) * NC],
                        start=(k == 0),
                        stop=(k == KT - 1),
                    )
            return ps

        def lphase(e1_t):
            # SBUF-only softplus tail + decay (2 table loads live here):
            # ln(e1+1), then decay = exp(-a * softplus).
            l1_t = []
            for j in range(ET):
                l1 = dpool.tile([P, NC], f32, name="l1", tag="l1", bufs=5)
                act(l1[:, :], e1_t[j][:, :], AF.Ln, bias=1.0)
                l1_t.append(l1)
            dec_t = []
            for j in range(ET):
                d = decpool.tile([P, NC], f32, name="dec", tag="dec")
                act(d[:, :], l1_t[j][:, :], AF.Exp, scale=na_sb[:, j:j + 1])
                dec_t.append(d)
            return dec_t

        BQ = ((1, spool, "tS"), (2, tpool, "tI"), (3, gpool, "tG"))

        def emit_front0(x_sb):
            # Chunk 0: K-OUTER blocks over 4 live psum banks per projection
            # so the PE consumes every k-slice the moment it lands.
            banksA = [(ppd, "ppd"), (ppd, "ppd"), (ppd, "ppd"), (pp, "pp")]
            banksB = [(pp, "pp"), (pp, "pp"), (pyp, "py"), (pyp, "py")]
            psA = mm_block_kouter(0, banksA, x_sb)
            e1_t = []
            for j in range(ET):
                e1 = dpool.tile([P, NC], f32, name="e1", tag="e1", bufs=5)
                act(e1[:, :], psA[j][:, :], AF.Exp)
                e1_t.append(e1)
            outs = []
            dec_t = None
            for qi, (q, pool, nm) in enumerate(BQ):
                psB = mm_block_kouter(q, banksB if qi % 2 == 0 else banksA,
                                      x_sb)
                ts = []
                for j in range(ET):
                    t = pool.tile([P, NC], f32, name=nm, tag=nm)
                    act(t[:, :], psB[j][:, :], AF.Tanh)
                    ts.append(t)
                outs.append(ts)
                if qi == 0:
                    dec_t = lphase(e1_t)
            front_state[0] = (dec_t, outs[0], outs[1], outs[2])

        def emit_front(c, x_sb):
            # Phase A: delta projection; Exp drains each psum as it fills.
            e1_t = []
            for j in range(ET):
                ps = ppd.tile([P, NC], f32, name="psd", tag="ppd")
                mm_group(ps, 0, j, x_sb)
                e1 = dpool.tile([P, NC], f32, name="e1", tag="e1", bufs=5)
                act(e1[:, :], ps[:, :], AF.Exp)
                e1_t.append(e1)
            # Phase B: the three gate projections, q-outer. The SBUF-only
            # Ln/decay block is emitted after the 6th Tanh, where the ACT
            # engine runs ~15us ahead of the PE's drain needs.
            outs = [[], [], []]
            dec_t = None
            nb = 0
            for qi, (q, pool, nm) in enumerate(BQ):
                for j in range(ET):
                    ps = pp.tile([P, NC], f32, name="ps", tag="pp")
                    mm_group(ps, q, j, x_sb)
                    t = pool.tile([P, NC], f32, name=nm, tag=nm)
                    act(t[:, :], ps[:, :], AF.Tanh)
                    outs[qi].append(t)
                    nb += 1
                    if nb == 6:
                        dec_t = lphase(e1_t)
            front_state[c] = (dec_t, outs[0], outs[1], outs[2])

        def phase_c(c):
            # update' = (1+tS)*tI ; scan ; gated' = (1+tG)*st  (bf16 out).
            dec_t, tS, tI, tG = front_state.pop(c)
            for j in range(ET):
                u = upool.tile([P, NC], f32, name="upd", tag="upd")
                nc.vector.scalar_tensor_tensor(
                    u[:, :], tS[j][:, :], 1.0, tI[j][:, :],
                    op0=OP.add, op1=OP.mult,
                )
                st = stpool.tile([P, NC], f32, name="st", tag="st")
                init = 0.0 if c == 0 else prev_states[j][:, NC - 1:NC]
                nc.vector.tensor_tensor_scan(
                    st[:, :], dec_t[j][:, :], u[:, :], init,
                    op0=OP.mult, op1=OP.add,
                )
                prev_states[j] = st
                g = gdpool.tile([P, NC], bf16, name="gated", tag="gated")
                nc.vector.scalar_tensor_tensor(
                    g[:, :], tG[j][:, :], 1.0, st[:, :],
                    op0=OP.add, op1=OP.mult,
                )
                gated_tiles[c][j] = g

        def emit_back(c):
            # Output GEMM for chunk c; psum->bf16 casts alternate between
            # Vector and Scalar so the pyp rotation never waits on a busy
            # Vector engine; one contiguous DMA per chunk.
            y_big = ypool.tile([P, MT * NC], bf16, name="y_big", tag="ybig")
            for m in range(MT):
                py = pyp.tile([P, NC], f32, name="py", tag="py")
                for j in range(KO):
                    nc.tensor.matmul(
                        py[:, :],
                        wo_sb[:, j * D + m * P: j * D + (m + 1) * P],
                        gated_tiles[c][j][:, :],
                        start=(j == 0),
                        stop=(j == KO - 1),
                    )
                dst = y_big[:, m * NC:(m + 1) * NC]
                if m % 2 == 0:
                    nc.vector.tensor_copy(dst, py[:, :])
                else:
                    nc.scalar.activation(dst, py[:, :], AF.Copy)
            nc.sync.dma_start(
                yt[:, c], y_big.rearrange("p (m s) -> p m s", m=MT))
            gated_tiles[c] = [None] * KO

        def emit_back_final(c):
            # Tail: last chunk's scan/gate/output GEMM in two 256-wide
            # halves; all 8 PSUM banks are free, accumulate j-major into 8
            # live half-bank psums; casts alternate Vector/Scalar; two
            # stores per half so the final flush overlaps the casts.
            dec_t, tS, tI, tG = front_state.pop(c)
            pools = [
                (pyp, "py"), (pyp, "py"), (pp, "pp"), (pp, "pp"),
                (pp, "pp"), (ppd, "ppd"), (ppd, "ppd"), (ppd, "ppd"),
            ]
            st_prev = prev_states
            for half in range(2):
                hsl = slice(half * HC, (half + 1) * HC)
                g_h = []
                for j in range(ET):
                    u = upool.tile([P, HC], f32, name="updh", tag="upd")
                    nc.vector.scalar_tensor_tensor(
                        u[:, :], tS[j][:, hsl], 1.0, tI[j][:, hsl],
                        op0=OP.add, op1=OP.mult,
                    )
                    st = stpool.tile([P, HC], f32, name="sth", tag="st")
                    init = st_prev[j][:, -1:]
                    nc.vector.tensor_tensor_scan(
                        st[:, :], dec_t[j][:, hsl], u[:, :], init,
                        op0=OP.mult, op1=OP.add,
                    )
                    st_prev[j] = st
                    g = gdpool.tile([P, HC], bf16, name="gatedh", tag="gated")
                    nc.vector.scalar_tensor_tensor(
                        g[:, :], tG[j][:, hsl], 1.0, st[:, :],
                        op0=OP.add, op1=OP.mult,
                    )
                    g_h.append(g)
                y_ps = [
                    pool.tile([P, HC], f32, name=f"pyf{m}", tag=tag)
                    for m, (pool, tag) in enumerate(pools)
                ]
                y_big = ypool.tile([P, MT * HC], bf16, name="y_bigh",
                                   tag="ybig")
                for j in range(KO - 1):
                    for m in range(MT):
                        nc.tensor.matmul(
                            y_ps[m][:, :],
                            wo_sb[:, j * D + m * P: j * D + (m + 1) * P],
                            g_h[j][:, :],
                            start=(j == 0),
                            stop=False,
                        )
                j = KO - 1
                for m in range(MT):
                    nc.tensor.matmul(
                        y_ps[m][:, :],
                        wo_sb[:, j * D + m * P: j * D + (m + 1) * P],
                        g_h[j][:, :],
                        start=False,
                        stop=True,
                    )
                    dst = y_big[:, m * HC:(m + 1) * HC]
                    if m % 2 == 0:
                        nc.vector.tensor_copy(dst, y_ps[m][:, :])
                    else:
                        nc.scalar.activation(dst, y_ps[m][:, :], AF.Copy)
                    if m == 3:
                        nc.sync.dma_start(
                            yt[:, c, 0:4, hsl],
                            y_big[:, 0:4 * HC].rearrange(
                                "p (m s) -> p m s", m=4))
                nc.sync.dma_start(
                    yt[:, c, 4:8, hsl],
                    y_big[:, 4 * HC:].rearrange("p (m s) -> p m s", m=4))

        for c in range(SC):
            x_cur = x_next
            if c == 0:
                x_next = x1
            elif c + 1 < SC:
                x_next = load_x(c + 1)
            if c == 0:
                emit_front0(x_cur)
            else:
                emit_front(c, x_cur)
            if c < SC - 1:
                phase_c(c)
            if c > 0:
                emit_back(c - 1)
        emit_back_final(SC - 1)

    nc.compile()
    return nc


def _get_program():
    if "nc" not in _CACHE:
        _CACHE["nc"] = _build_program()
    return _CACHE["nc"]


def _make_in_maps(x, W_in, W_select, W_gate, W_out, W_delta, log_a):
    import ml_dtypes

    bf = ml_dtypes.bfloat16
    a = (1.0 / (1.0 + np.exp(-log_a.astype(np.float32)))).astype(np.float32)
    in_maps = []
    for c in range(N_CORES):
        b, h = divmod(c, 2)
        sl = slice(h * E, (h + 1) * E)
        # xt[p, c, k, n] = x[b, c*NC+n, k*P+p]
        xT = np.ascontiguousarray(
            x[b].T.astype(bf).reshape(KT, P, SC, NC).transpose(1, 2, 0, 3)
        )                                                       # [P,SC,KT,NC]
        # w4[q, p, k, e] = Wq_eff.T[k*P+p, e]
        w4 = np.ascontiguousarray(
            np.stack(
                [
                    W_delta[sl, :].T,
                    0.5 * W_select[sl, :].T,   # sigmoid via tanh(z/2)
                    W_in[sl, :].T,
                    0.5 * W_gate[sl, :].T,     # sigmoid via tanh(z/2)
                ]
            ).astype(bf).reshape(4, KT, P, E).transpose(0, 2, 1, 3)
        )
        # wo[p, j*D+d] = Wo_eff.T[j*P+p, d]
        wo = np.ascontiguousarray(
            (0.25 * W_out[:, sl].T).astype(bf)
            .reshape(KO, P, D).transpose(1, 0, 2).reshape(P, KO * D)
        )
        na_m = np.ascontiguousarray((-a[sl]).reshape(ET, P).T)  # [P, ET]
        in_maps.append({"xt": xT, "w4": w4, "wo": wo, "na": na_m})
    return in_maps


def _gather(results):
    y = np.empty((B, S, D), np.float32)
    for b in range(B):
        yT = np.zeros((D, S), np.float32)
        for r in (results[2 * b], results[2 * b + 1]):
            # yt[p, c, m, n] = y_T[m*P+p, c*NC+n]
            arr = r["yt"].astype(np.float32)
            yT += arr.transpose(2, 0, 1, 3).reshape(D, S)
        y[b] = yT.T
    return y


def kernel(x, W_in, W_select, W_gate, W_out, W_delta, log_a):
    from concourse.bass_utils import run_bass_kernel_spmd

    nc = _get_program()
    in_maps = _make_in_maps(
        np.asarray(x, np.float32),
        np.asarray(W_in, np.float32),
        np.asarray(W_select, np.float32),
        np.asarray(W_gate, np.float32),
        np.asarray(W_out, np.float32),
        np.asarray(W_delta, np.float32),
        np.asarray(log_a, np.float32),
    )
    res = run_bass_kernel_spmd(nc, in_maps, core_ids=list(range(N_CORES)))
    return _gather(res.results)


if __name__ == "__main__":
    nc = _get_program()
    print("program built OK")


# revision 7
# speedup vs baseline: 1.0103x; 1.0103x over previous
"""ChaosSSMCore (diag mode) Trainium2 kernel.

Problem: B=4, S=4096, D=1024, fp32.
    delta  = softplus(x @ Wd.T); decay = exp(-delta * sigmoid(log_a))
    update = sigmoid(x @ Ws.T) * tanh(x @ Wi.T)
    gate   = sigmoid(x @ Wg.T)
    state_t = decay_t * state_{t-1} + update_t        (scan over S, elementwise in D)
    y      = (gate * states) @ Wo.T

Sharding: 8 cores = 4 batches x 2 halves of the D output dim. Each core
computes its 512-channel slice of the 4 input projections in [e, s] layout
(channels on partitions, sequence on the free axis), applies activations on
the scalar engine, runs the hardware tensor_tensor_scan (state = d*s + u
along the free dim) on the vector engine, gates, and computes a partial
output GEMM against its 512 columns of W_out. Host sums the two partials
per batch and transposes back.

All matmul operands are bf16 (host-side conversion; PSUM accumulation stays
fp32): the PE streams bf16 and fp32r at the same 1 row/cycle, but bf16
weight loads are shorter (less unhidden LDWEIGHTS time) and every input DMA
halves. Measured end-to-end rel err ~4e-3 vs the 2e-2 gate. The scan runs
in fp32; only the gated scan output (the moving operand of the output GEMM)
and the y partials written back to HBM are bf16.

DMA: aggregate bandwidth measures ~175GB/s no matter how many queues are
used, so startup is choreographed around strict priority: w0/x0 stream as
interleaved 128KB k-slices (sync/gpsimd queues), then w1,w2,w3 as k-slices,
then x(1), wo. Chunk 0's four projection blocks run K-OUTER across four
live psum banks so the PE consumes each arriving slice with 4 matmuls
instead of waiting for whole tensors, and a burst of dummy matmuls on
zeroed scratch warms the PE DVFS ramp before the first slice lands. y is
written back with one DMA per sequence chunk from a batched [P, MT*NC]
tile; psum->bf16 casts alternate Vector/Scalar so the output-GEMM psum
rotation never waits on a busy Vector engine.

Activation tables: sigmoid is computed as (1 + tanh(z/2))/2 with the 1/2
input scale folded into W_select/W_gate and the output scales into W_out
(W_out/4), so every psum-draining activation (delta-Exp + 12 Tanh) lives in
the `exp_and_others` table set. softplus(z) = ln(exp(z)+1) needs Ln from a
different set: the [4x Ln][4x decay-Exp] block (SBUF-only, 2 table loads)
is tucked into the middle of the Tanh phase where the ACT engine runs far
ahead of the PE's psum-drain needs, so the PE never waits on a table
switch. The ACT order is pinned chunk by chunk.

The last chunk's scan/gate/output GEMM run in two 256-wide halves with the
casts split across Vector and Scalar and two stores per half, so the final
HBM flush shrinks and starts earlier.
"""

import sys

if "/opt/trn_rl_repo" not in sys.path:
    sys.path.insert(0, "/opt/trn_rl_repo")

import numpy as np

# Problem constants (hardcoded per harness contract).
B, S, D = 4, 4096, 1024
P = 128           # SBUF partitions
E = D // 2        # per-core channel slice
NC = 512          # sequence chunk (= one PSUM bank of fp32)
KT = D // P       # k-tiles per input projection contraction (8)
ET = E // P       # e-tiles per core (4)
KO = E // P       # k'-tiles for the output GEMM contraction (4)
MT = D // P       # output-row tiles (8)
SC = S // NC      # sequence chunks (8)
HC = NC // 2      # half-chunk for the tail drain (256)
N_WARM = 11       # dummy warmup matmuls (PE DVFS ramp during startup DMAs)
N_CORES = 8

_CACHE = {}


def _build_program():
    import concourse.bacc as bacc
    import concourse.mybir as mybir
    import concourse.tile as tile
    from concourse.tile import add_dep_helper
    from contextlib import ExitStack

    f32 = mybir.dt.float32
    bf16 = mybir.dt.bfloat16
    AF = mybir.ActivationFunctionType
    OP = mybir.AluOpType

    nc = bacc.Bacc("TRN2", target_bir_lowering=False)

    # Host pre-arranges everything partition-major so each partition's slice
    # of any single DMA is contiguous DRAM.
    xt = nc.declare_dram_parameter("xt", [P, SC, KT, NC], bf16, isOutput=False)
    w4 = nc.declare_dram_parameter("w4", [4, P, KT, E], bf16, isOutput=False)
    wo = nc.declare_dram_parameter("wo", [P, KO * D], bf16, isOutput=False)
    na = nc.declare_dram_parameter("na", [P, ET], f32, isOutput=False)
    yt = nc.declare_dram_parameter("yt", [P, SC, MT, NC], bf16, isOutput=True)

    with tile.TileContext(nc) as tc, ExitStack() as ctx:
        wpool = ctx.enter_context(tc.tile_pool(name="w", bufs=1))
        xpool = ctx.enter_context(tc.tile_pool(name="x", bufs=2))
        ppd = ctx.enter_context(tc.tile_pool(name="ppd", bufs=3, space="PSUM"))
        pp = ctx.enter_context(tc.tile_pool(name="pp", bufs=3, space="PSUM"))
        pyp = ctx.enter_context(tc.tile_pool(name="pyp", bufs=2, space="PSUM"))
        dpool = ctx.enter_context(tc.tile_pool(name="dp", bufs=2))
        decpool = ctx.enter_context(tc.tile_pool(name="dec", bufs=8))
        spool = ctx.enter_context(tc.tile_pool(name="sp", bufs=4))
        tpool = ctx.enter_context(tc.tile_pool(name="tp", bufs=4))
        upool = ctx.enter_context(tc.tile_pool(name="up", bufs=3))
        stpool = ctx.enter_context(tc.tile_pool(name="stp", bufs=6))
        gpool = ctx.enter_context(tc.tile_pool(name="gp", bufs=4))
        gdpool = ctx.enter_context(tc.tile_pool(name="gdp", bufs=8))
        ypool = ctx.enter_context(tc.tile_pool(name="yp", bufs=2))

        # Pin the ACT instruction order to the emission order so the
        # scheduler cannot move a psum-draining activation behind a
        # table-switching phase from another chunk.
        last_act = [None]

        def act(*args, **kwargs):
            h = nc.scalar.activation(*args, **kwargs)
            if last_act[0] is not None:
                add_dep_helper(h.ins, last_act[0].ins, sync=False,
                               reason="pin ACT table phase order")
            last_act[0] = h
            return h

        # PE warmup scratch.
        xz = wpool.tile([P, NC], bf16, name="xz", tag="xz")
        wz = wpool.tile([P, P], bf16, name="wz", tag="wz")
        nc.vector.memset(xz[:, :], 0.0)
        nc.vector.memset(wz[:, :], 0.0)

        # Startup DMA choreography (shared ~175GB/s pipe, strict priority):
        #   gpsimd: na, x0 k-slices          (x0 paired with w0 stream)
        #   sync:   w0 k-slices, w1/w2/w3 k-slices, x1 k-slices, wo
        w_sb = [None] * 4
        for q in range(4):
            w_sb[q] = wpool.tile([P, KT * E], bf16, name=f"w{q}_sb", tag=f"w{q}")
        x_next = xpool.tile([P, KT * NC], bf16, name="x_sb", tag="x")
        na_sb = wpool.tile([P, ET], f32, name="na_sb", tag="na")
        wo_sb = wpool.tile([P, KO * D], bf16, name="wo_sb", tag="wo")

        for k in range(KT):
            nc.sync.dma_start(w_sb[0][:, k * E:(k + 1) * E], w4[0, :, k])
            nc.gpsimd.dma_start(
                x_next[:, k * NC:(k + 1) * NC], xt[:, 0, k])
        nc.gpsimd.dma_start(na_sb[:, :], na[:, :])
        for q in (1, 2, 3):
            for k in range(KT):
                nc.sync.dma_start(w_sb[q][:, k * E:(k + 1) * E], w4[q, :, k])
        x1 = xpool.tile([P, KT * NC], bf16, name="x_sb", tag="x")
        nc.sync.dma_start(x1.rearrange("p (k s) -> p k s", k=KT), xt[:, 1])
        nc.sync.dma_start(wo_sb[:, :], wo[:, :])

        def load_x(c):
            x_sb = xpool.tile([P, KT * NC], bf16, name="x_sb", tag="x")
            nc.gpsimd.dma_start(
                x_sb.rearrange("p (k s) -> p k s", k=KT), xt[:, c])
            return x_sb

        # Dummy warmup matmuls (zeros in, never read back).
        warm_ps = ppd.tile([P, NC], f32, name="warm", tag="ppd")
        for _ in range(N_WARM):
            nc.tensor.matmul(warm_ps[:, :], wz[:, :], xz[:, :],
                             start=True, stop=True)

        prev_states = [None] * ET
        gated_tiles = [[None] * ET for _ in range(SC)]
        front_state = {}

        def mm_group(ps, q, j, x_sb):
            for k in range(KT):
                nc.tensor.matmul(
                    ps[:, :],
                    w_sb[q][:, k * E + j * P: k * E + (j + 1) * P],
                    x_sb[:, k * NC:(k + 1) * NC],
                    start=(k == 0),
                    stop=(k == KT - 1),
                )

        def mm_block_kouter(q, banks, x_sb):
            ps = [pool.tile([P, NC], f32, name=f"ps0_{q}{j}", tag=tag)
                  for j, (pool, tag) in enumerate(banks)]
            for k in range(KT):
                for j in range(ET):
                    nc.tensor.matmul(
                        ps[j][:, :],
                        w_sb[q][:, k * E + j * P: k * E + (j + 1) * P],
                        x_sb[:, k * NC:(k + 1) * NC],
                        start=(k == 0),
                        stop=(k == KT - 1),
                    )
            return ps

        def lphase(e1_t):
            # SBUF-only softplus tail + decay (2 table loads live here):
            # ln(e1+1), then decay = exp(-a * softplus).
            l1_t = []
            for j in range(ET):
                l1 = dpool.tile([P, NC], f32, name="l1", tag="l1", bufs=5)
                act(l1[:, :], e1_t[j][:, :], AF.Ln, bias=1.0)
                l1_t.append(l1)
            dec_t = []
            for j in range(ET):
                d = decpool.tile([P, NC], f32, name="dec", tag="dec")
                act(d[:, :], l1_t[j][:, :], AF.Exp, scale=na_sb[:, j:j + 1])
                dec_t.append(d)
            return dec_t

        BQ = ((1, spool, "tS"), (2, tpool, "tI"), (3, gpool, "tG"))

        def emit_front0(x_sb):
            # Chunk 0: K-OUTER blocks over 4 live psum banks per projection
            # so the PE consumes every k-slice the moment it lands.
            banksA = [(ppd, "ppd"), (ppd, "ppd"), (ppd, "ppd"), (pp, "pp")]
            banksB = [(pp, "pp"), (pp, "pp"), (pyp, "py"), (pyp, "py")]
            psA = mm_block_kouter(0, banksA, x_sb)
            e1_t = []
            for j in range(ET):
                e1 = dpool.tile([P, NC], f32, name="e1", tag="e1", bufs=5)
                act(e1[:, :], psA[j][:, :], AF.Exp)
                e1_t.append(e1)
            outs = []
            dec_t = None
            for qi, (q, pool, nm) in enumerate(BQ):
                psB = mm_block_kouter(q, banksB if qi % 2 == 0 else banksA,
                                      x_sb)
                ts = []
                for j in range(ET):
                    t = pool.tile([P, NC], f32, name=nm, tag=nm)
                    act(t[:, :], psB[j][:, :], AF.Tanh)
                    ts.append(t)
                outs.append(ts)
                if qi == 0:
                    dec_t = lphase(e1_t)
            front_state[0] = (dec_t, outs[0], outs[1], outs[2])

        def emit_front(c, x_sb):
            # Phase A: delta projection; Exp drains each psum as it fills.
            e1_t = []
            for j in range(ET):
                ps = ppd.tile([P, NC], f32, name="psd", tag="ppd")
                mm_group(ps, 0, j, x_sb)
                e1 = dpool.tile([P, NC], f32, name="e1", tag="e1", bufs=5)
                act(e1[:, :], ps[:, :], AF.Exp)
                e1_t.append(e1)
            # Phase B: the three gate projections, q-outer. The SBUF-only
            # Ln/decay block is emitted after the 6th Tanh, where the ACT
            # engine runs ~15us ahead of the PE's drain needs.
            outs = [[], [], []]
            dec_t = None
            nb = 0
            for qi, (q, pool, nm) in enumerate(BQ):
                for j in range(ET):
                    ps = pp.tile([P, NC], f32, name="ps", tag="pp")
                    mm_group(ps, q, j, x_sb)
                    t = pool.tile([P, NC], f32, name=nm, tag=nm)
                    act(t[:, :], ps[:, :], AF.Tanh)
                    outs[qi].append(t)
                    nb += 1
                    if nb == 6:
                        dec_t = lphase(e1_t)
            front_state[c] = (dec_t, outs[0], outs[1], outs[2])

        def phase_c(c):
            # update' = (1+tS)*tI ; scan ; gated' = (1+tG)*st  (bf16 out).
            dec_t, tS, tI, tG = front_state.pop(c)
            for j in range(ET):
                u = upool.tile([P, NC], f32, name="upd", tag="upd")
                nc.vector.scalar_tensor_tensor(
                    u[:, :], tS[j][:, :], 1.0, tI[j][:, :],
                    op0=OP.add, op1=OP.mult,
                )
                st = stpool.tile([P, NC], f32, name="st", tag="st")
                init = 0.0 if c == 0 else prev_states[j][:, NC - 1:NC]
                nc.vector.tensor_tensor_scan(
                    st[:, :], dec_t[j][:, :], u[:, :], init,
                    op0=OP.mult, op1=OP.add,
                )
                prev_states[j] = st
                g = gdpool.tile([P, NC], bf16, name="gated", tag="gated")
                nc.vector.scalar_tensor_tensor(
                    g[:, :], tG[j][:, :], 1.0, st[:, :],
                    op0=OP.add, op1=OP.mult,
                )
                gated_tiles[c][j] = g

        def emit_back(c):
            # Output GEMM for chunk c; psum->bf16 casts alternate between
            # Vector and Scalar so the pyp rotation never waits on a busy
            # Vector engine; one contiguous DMA per chunk.
            y_big = ypool.tile([P, MT * NC], bf16, name="y_big", tag="ybig")
            for m in range(MT):
                py = pyp.tile([P, NC], f32, name="py", tag="py")
                for j in range(KO):
                    nc.tensor.matmul(
                        py[:, :],
                        wo_sb[:, j * D + m * P: j * D + (m + 1) * P],
                        gated_tiles[c][j][:, :],
                        start=(j == 0),
                        stop=(j == KO - 1),
                    )
                dst = y_big[:, m * NC:(m + 1) * NC]
                if m % 2 == 0:
                    nc.vector.tensor_copy(dst, py[:, :])
                else:
                    nc.scalar.activation(dst, py[:, :], AF.Copy)
            nc.sync.dma_start(
                yt[:, c], y_big.rearrange("p (m s) -> p m s", m=MT))
            gated_tiles[c] = [None] * KO

        def emit_back_final(c):
            # Tail: last chunk's scan/gate/output GEMM in two 256-wide
            # halves; all 8 PSUM banks are free, accumulate j-major into 8
            # live half-bank psums; casts alternate Vector/Scalar; two
            # stores per half so the final flush overlaps the casts.
            dec_t, tS, tI, tG = front_state.pop(c)
            pools = [
                (pyp, "py"), (pyp, "py"), (pp, "pp"), (pp, "pp"),
                (pp, "pp"), (ppd, "ppd"), (ppd, "ppd"), (ppd, "ppd"),
            ]
            st_prev = prev_states
            for half in range(2):
                hsl = slice(half * HC, (half + 1) * HC)
                g_h = []
                for j in range(ET):
                    u = upool.tile([P, HC], f32, name="updh", tag="upd")
                    nc.vector.scalar_tensor_tensor(
                        u[:, :], tS[j][:, hsl], 1.0, tI[j][:, hsl],
                        op0=OP.add, op1=OP.mult,
                    )
                    st = stpool.tile([P, HC], f32, name="sth", tag="st")
                    init = st_prev[j][:, -1:]
                    nc.vector.tensor_tensor_scan(
                        st[:, :], dec_t[j][:, hsl], u[:, :], init,
                        op0=OP.mult, op1=OP.add,
                    )
                    st_prev[j] = st
                    g = gdpool.tile([P, HC], bf16, name="gatedh", tag="gated")
                    nc.vector.scalar_tensor_tensor(
                        g[:, :], tG[j][:, hsl], 1.0, st[:, :],
                        op0=OP.add, op1=OP.mult,
                    )
                    g_h.append(g)
                y_ps = [
                    pool.tile([P, HC], f32, name=f"pyf{m}", tag=tag)
                    for m, (pool, tag) in enumerate(pools)
                ]
                y_big = ypool.tile([P, MT * HC], bf16, name="y_bigh",
                                   tag="ybig")
                for j in range(KO - 1):
                    for m in range(MT):
                        nc.tensor.matmul(
                            y_ps[m][:, :],
                            wo_sb[:, j * D + m * P: j * D + (m + 1) * P],
                            g_h[j][:, :],
                            start=(j == 0),
                            stop=False,
                        )
                j = KO - 1
                for m in range(MT):
                    nc.tensor.matmul(
                        y_ps[m][:, :],
                        wo_sb[:, j * D + m * P: j * D + (m + 1) * P],
                        g_h[j][:, :],
                        start=False,
                        stop=True,
                    )
                    dst = y_big[:, m * HC:(m + 1) * HC]
                    if m % 2 == 0:
                        nc.vector.tensor_copy(dst, y_ps[m][:, :])
                    else:
                        nc.scalar.activation(dst, y_ps[m][:, :], AF.Copy)
                    if m == 3:
                        nc.sync.dma_start(
                            yt[:, c, 0:4, hsl],
                            y_big[:, 0:4 * HC].rearrange(
                                "p (m s) -> p m s", m=4))
                nc.sync.dma_start(
                    yt[:, c, 4:8, hsl],
                    y_big[:, 4 * HC:].rearrange("p (m s) -> p m s", m=4))

        for c in range(SC):
            x_cur = x_next
            if c == 0:
                x_next = x1
            elif c + 1 < SC:
                x_next = load_x(c + 1)
            if c == 0:
                emit_front0(x_cur)
            else:
                emit_front(c, x_cur)
            # Emission order matters for the Vector queue: back(c-1)'s
            # casts must queue BEFORE phase_c(c)'s scan chain, or the
            # output-GEMM psum rotation waits on the whole chain.
            if c > 0:
                emit_back(c - 1)
            if c < SC - 1:
                phase_c(c)
        emit_back_final(SC - 1)

    nc.compile()
    return nc


def _get_program():
    if "nc" not in _CACHE:
        _CACHE["nc"] = _build_program()
    return _CACHE["nc"]


def _make_in_maps(x, W_in, W_select, W_gate, W_out, W_delta, log_a):
    import ml_dtypes

    bf = ml_dtypes.bfloat16
    a = (1.0 / (1.0 + np.exp(-log_a.astype(np.float32)))).astype(np.float32)
    in_maps = []
    for c in range(N_CORES):
        b, h = divmod(c, 2)
        sl = slice(h * E, (h + 1) * E)
        # xt[p, c, k, n] = x[b, c*NC+n, k*P+p]
        xT = np.ascontiguousarray(
            x[b].T.astype(bf).reshape(KT, P, SC, NC).transpose(1, 2, 0, 3)
        )                                                       # [P,SC,KT,NC]
        # w4[q, p, k, e] = Wq_eff.T[k*P+p, e]
        w4 = np.ascontiguousarray(
            np.stack(
                [
                    W_delta[sl, :].T,
                    0.5 * W_select[sl, :].T,   # sigmoid via tanh(z/2)
                    W_in[sl, :].T,
                    0.5 * W_gate[sl, :].T,     # sigmoid via tanh(z/2)
                ]
            ).astype(bf).reshape(4, KT, P, E).transpose(0, 2, 1, 3)
        )
        # wo[p, j*D+d] = Wo_eff.T[j*P+p, d]
        wo = np.ascontiguousarray(
            (0.25 * W_out[:, sl].T).astype(bf)
            .reshape(KO, P, D).transpose(1, 0, 2).reshape(P, KO * D)
        )
        na_m = np.ascontiguousarray((-a[sl]).reshape(ET, P).T)  # [P, ET]
        in_maps.append({"xt": xT, "w4": w4, "wo": wo, "na": na_m})
    return in_maps


def _gather(results):
    y = np.empty((B, S, D), np.float32)
    for b in range(B):
        yT = np.zeros((D, S), np.float32)
        for r in (results[2 * b], results[2 * b + 1]):
            # yt[p, c, m, n] = y_T[m*P+p, c*NC+n]
            arr = r["yt"].astype(np.float32)
            yT += arr.transpose(2, 0, 1, 3).reshape(D, S)
        y[b] = yT.T
    return y


def kernel(x, W_in, W_select, W_gate, W_out, W_delta, log_a):
    from concourse.bass_utils import run_bass_kernel_spmd

    nc = _get_program()
    in_maps = _make_in_maps(
        np.asarray(x, np.float32),
        np.asarray(W_in, np.float32),
        np.asarray(W_select, np.float32),
        np.asarray(W_gate, np.float32),
        np.asarray(W_out, np.float32),
        np.asarray(W_delta, np.float32),
        np.asarray(log_a, np.float32),
    )
    res = run_bass_kernel_spmd(nc, in_maps, core_ids=list(range(N_CORES)))
    return _gather(res.results)


if __name__ == "__main__":
    nc = _get_program()
    print("program built OK")


# revision 9
# speedup vs baseline: 1.0831x; 1.0720x over previous
"""ChaosSSMCore (diag mode) Trainium2 kernel.

Problem: B=4, S=4096, D=1024, fp32.
    delta  = softplus(x @ Wd.T); decay = exp(-delta * sigmoid(log_a))
    update = sigmoid(x @ Ws.T) * tanh(x @ Wi.T)
    gate   = sigmoid(x @ Wg.T)
    state_t = decay_t * state_{t-1} + update_t        (scan over S, elementwise in D)
    y      = (gate * states) @ Wo.T

Sharding: 8 cores = 4 batches x 2 halves of the D output dim. Each core
computes its 512-channel slice of the 4 input projections in [e, s] layout
(channels on partitions, sequence on the free axis), applies activations on
the scalar engine, runs the hardware tensor_tensor_scan (state = d*s + u
along the free dim) on the vector engine, gates, and computes a partial
output GEMM against its 512 columns of W_out. Host sums the two partials
per batch and transposes back.

All matmul operands are bf16 (host-side conversion; PSUM accumulation stays
fp32): the PE streams bf16 and fp32r at the same 1 row/cycle, but bf16
weight loads are shorter (less unhidden LDWEIGHTS time) and every input DMA
halves. Measured end-to-end rel err ~4e-3 vs the 2e-2 gate. The scan runs
in fp32; only the gated scan output (the moving operand of the output GEMM)
and the y partials written back to HBM are bf16.

DMA: aggregate bandwidth measures ~175GB/s no matter how many queues are
used, so startup is choreographed around strict priority: w0/x0 stream as
interleaved 128KB k-slices (sync/gpsimd queues), then w1,w2,w3 as k-slices,
then x(1), wo. Chunk 0's four projection blocks run K-OUTER across four
live psum banks so the PE consumes each arriving slice with 4 matmuls
instead of waiting for whole tensors, and a burst of dummy matmuls on
zeroed scratch warms the PE DVFS ramp before the first slice lands. y is
written back with one DMA per sequence chunk from a batched [P, MT*NC]
tile; psum->bf16 casts alternate Vector/Scalar so the output-GEMM psum
rotation never waits on a busy Vector engine.

Activation tables: sigmoid is computed as (1 + tanh(z/2))/2 with the 1/2
input scale folded into W_select/W_gate and the output scales into W_out
(W_out/4), so every psum-draining activation (delta-Exp + 12 Tanh) lives in
the `exp_and_others` table set. softplus(z) = ln(exp(z)+1) needs Ln from a
different set: the [4x Ln][4x decay-Exp] block (SBUF-only, 2 table loads)
is tucked into the middle of the Tanh phase where the ACT engine runs far
ahead of the PE's psum-drain needs, so the PE never waits on a table
switch. The ACT order is pinned chunk by chunk.

The last chunk's scan/gate/output GEMM run in two 256-wide halves with the
casts split across Vector and Scalar and two stores per half, so the final
HBM flush shrinks and starts earlier.
"""

import sys

if "/opt/trn_rl_repo" not in sys.path:
    sys.path.insert(0, "/opt/trn_rl_repo")

import numpy as np

# Problem constants (hardcoded per harness contract).
B, S, D = 4, 4096, 1024
P = 128           # SBUF partitions
E = D // 2        # per-core channel slice
NC = 512          # sequence chunk (= one PSUM bank of fp32)
KT = D // P       # k-tiles per input projection contraction (8)
ET = E // P       # e-tiles per core (4)
KO = E // P       # k'-tiles for the output GEMM contraction (4)
MT = D // P       # output-row tiles (8)
SC = S // NC      # sequence chunks (8)
HC = NC // 2      # half-chunk for the tail drain (256)
N_WARM = 11       # dummy warmup matmuls (PE DVFS ramp during startup DMAs)
N_CORES = 8

_CACHE = {}


def _build_program():
    import concourse.bacc as bacc
    import concourse.mybir as mybir
    import concourse.tile as tile
    from concourse.tile import add_dep_helper
    from contextlib import ExitStack

    f32 = mybir.dt.float32
    bf16 = mybir.dt.bfloat16
    AF = mybir.ActivationFunctionType
    OP = mybir.AluOpType

    nc = bacc.Bacc("TRN2", target_bir_lowering=False)

    # Host pre-arranges everything partition-major so each partition's slice
    # of any single DMA is contiguous DRAM.
    xt = nc.declare_dram_parameter("xt", [P, SC, KT, NC], bf16, isOutput=False)
    w4 = nc.declare_dram_parameter("w4", [4, P, KT, E], bf16, isOutput=False)
    wo = nc.declare_dram_parameter("wo", [P, KO * D], bf16, isOutput=False)
    na = nc.declare_dram_parameter("na", [P, ET], f32, isOutput=False)
    yt = nc.declare_dram_parameter("yt", [P, SC, MT, NC], bf16, isOutput=True)

    with tile.TileContext(nc) as tc, ExitStack() as ctx:
        wpool = ctx.enter_context(tc.tile_pool(name="w", bufs=1))
        xpool = ctx.enter_context(tc.tile_pool(name="x", bufs=2))
        ppd = ctx.enter_context(tc.tile_pool(name="ppd", bufs=3, space="PSUM"))
        pp = ctx.enter_context(tc.tile_pool(name="pp", bufs=3, space="PSUM"))
        pyp = ctx.enter_context(tc.tile_pool(name="pyp", bufs=2, space="PSUM"))
        dpool = ctx.enter_context(tc.tile_pool(name="dp", bufs=2))
        decpool = ctx.enter_context(tc.tile_pool(name="dec", bufs=8))
        spool = ctx.enter_context(tc.tile_pool(name="sp", bufs=4))
        tpool = ctx.enter_context(tc.tile_pool(name="tp", bufs=4))
        upool = ctx.enter_context(tc.tile_pool(name="up", bufs=3))
        stpool = ctx.enter_context(tc.tile_pool(name="stp", bufs=6))
        gpool = ctx.enter_context(tc.tile_pool(name="gp", bufs=4))
        gdpool = ctx.enter_context(tc.tile_pool(name="gdp", bufs=8))
        ypool = ctx.enter_context(tc.tile_pool(name="yp", bufs=2))

        # Pin the ACT instruction order to the emission order so the
        # scheduler cannot move a psum-draining activation behind a
        # table-switching phase from another chunk.
        last_act = [None]

        def act(*args, **kwargs):
            h = nc.scalar.activation(*args, **kwargs)
            if last_act[0] is not None:
                add_dep_helper(h.ins, last_act[0].ins, sync=False,
                               reason="pin ACT table phase order")
            last_act[0] = h
            return h

        # PE warmup scratch.
        xz = wpool.tile([P, NC], bf16, name="xz", tag="xz")
        wz = wpool.tile([P, P], bf16, name="wz", tag="wz")
        nc.vector.memset(xz[:, :], 0.0)
        nc.vector.memset(wz[:, :], 0.0)

        # Startup DMA choreography (shared ~175GB/s pipe, strict priority):
        #   gpsimd: na, x0 k-slices          (x0 paired with w0 stream)
        #   sync:   w0 k-slices, w1/w2/w3 k-slices, x1 k-slices, wo
        w_sb = [None] * 4
        for q in range(4):
            w_sb[q] = wpool.tile([P, KT * E], bf16, name=f"w{q}_sb", tag=f"w{q}")
        x_next = xpool.tile([P, KT * NC], bf16, name="x_sb", tag="x")
        na_sb = wpool.tile([P, ET], f32, name="na_sb", tag="na")
        wo_sb = wpool.tile([P, KO * D], bf16, name="wo_sb", tag="wo")

        for k in range(KT):
            nc.sync.dma_start(w_sb[0][:, k * E:(k + 1) * E], w4[0, :, k])
            nc.gpsimd.dma_start(
                x_next[:, k * NC:(k + 1) * NC], xt[:, 0, k])
        nc.gpsimd.dma_start(na_sb[:, :], na[:, :])
        for q in (1, 2, 3):
            for k in range(KT):
                nc.sync.dma_start(w_sb[q][:, k * E:(k + 1) * E], w4[q, :, k])
        x1 = xpool.tile([P, KT * NC], bf16, name="x_sb", tag="x")
        nc.sync.dma_start(x1.rearrange("p (k s) -> p k s", k=KT), xt[:, 1])
        nc.sync.dma_start(wo_sb[:, :], wo[:, :])

        def load_x(c):
            x_sb = xpool.tile([P, KT * NC], bf16, name="x_sb", tag="x")
            nc.gpsimd.dma_start(
                x_sb.rearrange("p (k s) -> p k s", k=KT), xt[:, c])
            return x_sb

        # Dummy warmup matmuls (zeros in, never read back).
        warm_ps = ppd.tile([P, NC], f32, name="warm", tag="ppd")
        for _ in range(N_WARM):
            nc.tensor.matmul(warm_ps[:, :], wz[:, :], xz[:, :],
                             start=True, stop=True)

        prev_states = [None] * ET
        gated_tiles = [[None] * ET for _ in range(SC)]
        front_state = {}

        def mm_group(ps, q, j, x_sb):
            for k in range(KT):
                nc.tensor.matmul(
                    ps[:, :],
                    w_sb[q][:, k * E + j * P: k * E + (j + 1) * P],
                    x_sb[:, k * NC:(k + 1) * NC],
                    start=(k == 0),
                    stop=(k == KT - 1),
                )

        def mm_block_kouter(q, banks, x_sb):
            ps = [pool.tile([P, NC], f32, name=f"ps0_{q}{j}", tag=tag)
                  for j, (pool, tag) in enumerate(banks)]
            for k in range(KT):
                for j in range(ET):
                    nc.tensor.matmul(
                        ps[j][:, :],
                        w_sb[q][:, k * E + j * P: k * E + (j + 1) * P],
                        x_sb[:, k * NC:(k + 1) * NC],
                        start=(k == 0),
                        stop=(k == KT - 1),
                    )
            return ps

        def lphase(e1_t):
            # SBUF-only softplus tail + decay (2 table loads live here):
            # ln(e1+1), then decay = exp(-a * softplus).
            l1_t = []
            for j in range(ET):
                l1 = dpool.tile([P, NC], f32, name="l1", tag="l1", bufs=5)
                act(l1[:, :], e1_t[j][:, :], AF.Ln, bias=1.0)
                l1_t.append(l1)
            dec_t = []
            for j in range(ET):
                d = decpool.tile([P, NC], f32, name="dec", tag="dec")
                act(d[:, :], l1_t[j][:, :], AF.Exp, scale=na_sb[:, j:j + 1])
                dec_t.append(d)
            return dec_t

        BQ = ((1, spool, "tS"), (2, tpool, "tI"), (3, gpool, "tG"))

        def emit_front0(x_sb):
            # Chunk 0: K-OUTER blocks over 4 live psum banks per projection
            # so the PE consumes every k-slice the moment it lands.
            banksA = [(ppd, "ppd"), (ppd, "ppd"), (ppd, "ppd"), (pp, "pp")]
            banksB = [(pp, "pp"), (pp, "pp"), (pyp, "py"), (pyp, "py")]
            psA = mm_block_kouter(0, banksA, x_sb)
            e1_t = []
            for j in range(ET):
                e1 = dpool.tile([P, NC], f32, name="e1", tag="e1", bufs=5)
                act(e1[:, :], psA[j][:, :], AF.Exp)
                e1_t.append(e1)
            outs = []
            dec_t = None
            for qi, (q, pool, nm) in enumerate(BQ):
                psB = mm_block_kouter(q, banksB if qi % 2 == 0 else banksA,
                                      x_sb)
                ts = []
                for j in range(ET):
                    t = pool.tile([P, NC], f32, name=nm, tag=nm)
                    act(t[:, :], psB[j][:, :], AF.Tanh)
                    ts.append(t)
                outs.append(ts)
                if qi == 0:
                    dec_t = lphase(e1_t)
            front_state[0] = (dec_t, outs[0], outs[1], outs[2])

        def emit_front(c, x_sb):
            # Phase A: delta projection; Exp drains each psum as it fills.
            e1_t = []
            for j in range(ET):
                ps = ppd.tile([P, NC], f32, name="psd", tag="ppd")
                mm_group(ps, 0, j, x_sb)
                e1 = dpool.tile([P, NC], f32, name="e1", tag="e1", bufs=5)
                act(e1[:, :], ps[:, :], AF.Exp)
                e1_t.append(e1)
            # SBUF-only Ln/decay block right after the delta drains (its 2
            # table loads run while the PE is still in the delta block's
            # shadow and the Tanh drains haven't started).
            dec_t = lphase(e1_t)
            # Phase B: the three gate projections, q-outer, cycling psum
            # banks across BOTH the pp and ppd pools (6-deep rotation, so
            # a B-group never waits on a Tanh drain only 3 groups back).
            outs = [[], [], []]
            nb = 0
            for qi, (q, pool, nm) in enumerate(BQ):
                for j in range(ET):
                    bp, btag = ((pp, "pp") if (nb // 3) % 2 == 0
                                else (ppd, "ppd"))
                    ps = bp.tile([P, NC], f32, name="ps", tag=btag)
                    mm_group(ps, q, j, x_sb)
                    t = pool.tile([P, NC], f32, name=nm, tag=nm)
                    act(t[:, :], ps[:, :], AF.Tanh)
                    outs[qi].append(t)
                    nb += 1
            front_state[c] = (dec_t, outs[0], outs[1], outs[2])

        def phase_c(c):
            # update' = (1+tS)*tI ; scan ; gated' = (1+tG)*st  (bf16 out).
            dec_t, tS, tI, tG = front_state.pop(c)
            for j in range(ET):
                u = upool.tile([P, NC], f32, name="upd", tag="upd")
                nc.vector.scalar_tensor_tensor(
                    u[:, :], tS[j][:, :], 1.0, tI[j][:, :],
                    op0=OP.add, op1=OP.mult,
                )
                st = stpool.tile([P, NC], f32, name="st", tag="st")
                init = 0.0 if c == 0 else prev_states[j][:, NC - 1:NC]
                nc.vector.tensor_tensor_scan(
                    st[:, :], dec_t[j][:, :], u[:, :], init,
                    op0=OP.mult, op1=OP.add,
                )
                prev_states[j] = st
                g = gdpool.tile([P, NC], bf16, name="gated", tag="gated")
                nc.vector.scalar_tensor_tensor(
                    g[:, :], tG[j][:, :], 1.0, st[:, :],
                    op0=OP.add, op1=OP.mult,
                )
                gated_tiles[c][j] = g

        def emit_back(c):
            # Output GEMM for chunk c; psum->bf16 casts alternate between
            # Vector and Scalar so the pyp rotation never waits on a busy
            # Vector engine; one contiguous DMA per chunk.
            y_big = ypool.tile([P, MT * NC], bf16, name="y_big", tag="ybig")
            for m in range(MT):
                py = pyp.tile([P, NC], f32, name="py", tag="py")
                for j in range(KO):
                    nc.tensor.matmul(
                        py[:, :],
                        wo_sb[:, j * D + m * P: j * D + (m + 1) * P],
                        gated_tiles[c][j][:, :],
                        start=(j == 0),
                        stop=(j == KO - 1),
                    )
                dst = y_big[:, m * NC:(m + 1) * NC]
                if m % 2 == 0:
                    nc.vector.tensor_copy(dst, py[:, :])
                else:
                    nc.scalar.activation(dst, py[:, :], AF.Copy)
            nc.sync.dma_start(
                yt[:, c], y_big.rearrange("p (m s) -> p m s", m=MT))
            gated_tiles[c] = [None] * KO

        def emit_back_final(c):
            # Tail: last chunk's scan/gate/output GEMM in two 256-wide
            # halves; all 8 PSUM banks are free, accumulate j-major into 8
            # live half-bank psums; casts alternate Vector/Scalar; two
            # stores per half so the final flush overlaps the casts.
            dec_t, tS, tI, tG = front_state.pop(c)
            pools = [
                (pyp, "py"), (pyp, "py"), (pp, "pp"), (pp, "pp"),
                (pp, "pp"), (ppd, "ppd"), (ppd, "ppd"), (ppd, "ppd"),
            ]
            st_prev = prev_states
            for half in range(2):
                hsl = slice(half * HC, (half + 1) * HC)
                g_h = []
                for j in range(ET):
                    u = upool.tile([P, HC], f32, name="updh", tag="upd")
                    nc.vector.scalar_tensor_tensor(
                        u[:, :], tS[j][:, hsl], 1.0, tI[j][:, hsl],
                        op0=OP.add, op1=OP.mult,
                    )
                    st = stpool.tile([P, HC], f32, name="sth", tag="st")
                    init = st_prev[j][:, -1:]
                    nc.vector.tensor_tensor_scan(
                        st[:, :], dec_t[j][:, hsl], u[:, :], init,
                        op0=OP.mult, op1=OP.add,
                    )
                    st_prev[j] = st
                    g = gdpool.tile([P, HC], bf16, name="gatedh", tag="gated")
                    nc.vector.scalar_tensor_tensor(
                        g[:, :], tG[j][:, hsl], 1.0, st[:, :],
                        op0=OP.add, op1=OP.mult,
                    )
                    g_h.append(g)
                y_ps = [
                    pool.tile([P, HC], f32, name=f"pyf{m}", tag=tag)
                    for m, (pool, tag) in enumerate(pools)
                ]
                y_big = ypool.tile([P, MT * HC], bf16, name="y_bigh",
                                   tag="ybig")
                for j in range(KO - 1):
                    for m in range(MT):
                        nc.tensor.matmul(
                            y_ps[m][:, :],
                            wo_sb[:, j * D + m * P: j * D + (m + 1) * P],
                            g_h[j][:, :],
                            start=(j == 0),
                            stop=False,
                        )
                j = KO - 1
                for m in range(MT):
                    nc.tensor.matmul(
                        y_ps[m][:, :],
                        wo_sb[:, j * D + m * P: j * D + (m + 1) * P],
                        g_h[j][:, :],
                        start=False,
                        stop=True,
                    )
                    dst = y_big[:, m * HC:(m + 1) * HC]
                    if m % 2 == 0:
                        nc.vector.tensor_copy(dst, y_ps[m][:, :])
                    else:
                        nc.scalar.activation(dst, y_ps[m][:, :], AF.Copy)
                    if m == 3:
                        nc.sync.dma_start(
                            yt[:, c, 0:4, hsl],
                            y_big[:, 0:4 * HC].rearrange(
                                "p (m s) -> p m s", m=4))
                nc.sync.dma_start(
                    yt[:, c, 4:8, hsl],
                    y_big[:, 4 * HC:].rearrange("p (m s) -> p m s", m=4))

        for c in range(SC):
            x_cur = x_next
            if c == 0:
                x_next = x1
            elif c + 1 < SC:
                x_next = load_x(c + 1)
            if c == 0:
                emit_front0(x_cur)
            else:
                emit_front(c, x_cur)
            if c < SC - 1:
                phase_c(c)
            if c > 0:
                emit_back(c - 1)
        emit_back_final(SC - 1)

    nc.compile()
    return nc


def _get_program():
    if "nc" not in _CACHE:
        _CACHE["nc"] = _build_program()
    return _CACHE["nc"]


def _make_in_maps(x, W_in, W_select, W_gate, W_out, W_delta, log_a):
    import ml_dtypes

    bf = ml_dtypes.bfloat16
    a = (1.0 / (1.0 + np.exp(-log_a.astype(np.float32)))).astype(np.float32)
    in_maps = []
    for c in range(N_CORES):
        b, h = divmod(c, 2)
        sl = slice(h * E, (h + 1) * E)
        # xt[p, c, k, n] = x[b, c*NC+n, k*P+p]
        xT = np.ascontiguousarray(
            x[b].T.astype(bf).reshape(KT, P, SC, NC).transpose(1, 2, 0, 3)
        )                                                       # [P,SC,KT,NC]
        # w4[q, p, k, e] = Wq_eff.T[k*P+p, e]
        w4 = np.ascontiguousarray(
            np.stack(
                [
                    W_delta[sl, :].T,
                    0.5 * W_select[sl, :].T,   # sigmoid via tanh(z/2)
                    W_in[sl, :].T,
                    0.5 * W_gate[sl, :].T,     # sigmoid via tanh(z/2)
                ]
            ).astype(bf).reshape(4, KT, P, E).transpose(0, 2, 1, 3)
        )
        # wo[p, j*D+d] = Wo_eff.T[j*P+p, d]
        wo = np.ascontiguousarray(
            (0.25 * W_out[:, sl].T).astype(bf)
            .reshape(KO, P, D).transpose(1, 0, 2).reshape(P, KO * D)
        )
        na_m = np.ascontiguousarray((-a[sl]).reshape(ET, P).T)  # [P, ET]
        in_maps.append({"xt": xT, "w4": w4, "wo": wo, "na": na_m})
    return in_maps


def _gather(results):
    y = np.empty((B, S, D), np.float32)
    for b in range(B):
        yT = np.zeros((D, S), np.float32)
        for r in (results[2 * b], results[2 * b + 1]):
            # yt[p, c, m, n] = y_T[m*P+p, c*NC+n]
            arr = r["yt"].astype(np.float32)
            yT += arr.transpose(2, 0, 1, 3).reshape(D, S)
        y[b] = yT.T
    return y


def kernel(x, W_in, W_select, W_gate, W_out, W_delta, log_a):
    from concourse.bass_utils import run_bass_kernel_spmd

    nc = _get_program()
    in_maps = _make_in_maps(
        np.asarray(x, np.float32),
        np.asarray(W_in, np.float32),
        np.asarray(W_select, np.float32),
        np.asarray(W_gate, np.float32),
        np.asarray(W_out, np.float32),
        np.asarray(W_delta, np.float32),
        np.asarray(log_a, np.float32),
    )
    res = run_bass_kernel_spmd(nc, in_maps, core_ids=list(range(N_CORES)))
    return _gather(res.results)


if __name__ == "__main__":
    nc = _get_program()
    print("program built OK")


# revision 11
# speedup vs baseline: 1.0889x; 1.0053x over previous
"""ChaosSSMCore (diag mode) Trainium2 kernel.

Problem: B=4, S=4096, D=1024, fp32.
    delta  = softplus(x @ Wd.T); decay = exp(-delta * sigmoid(log_a))
    update = sigmoid(x @ Ws.T) * tanh(x @ Wi.T)
    gate   = sigmoid(x @ Wg.T)
    state_t = decay_t * state_{t-1} + update_t        (scan over S, elementwise in D)
    y      = (gate * states) @ Wo.T

Sharding: 8 cores = 4 batches x 2 halves of the D output dim. Each core
computes its 512-channel slice of the 4 input projections in [e, s] layout
(channels on partitions, sequence on the free axis), applies activations on
the scalar engine, runs the hardware tensor_tensor_scan (state = d*s + u
along the free dim) on the vector engine, gates, and computes a partial
output GEMM against its 512 columns of W_out. Host sums the two partials
per batch and transposes back.

All matmul operands are bf16 (host-side conversion; PSUM accumulation stays
fp32): the PE streams bf16 and fp32r at the same 1 row/cycle, but bf16
weight loads are shorter (less unhidden LDWEIGHTS time) and every input DMA
halves. Measured end-to-end rel err ~4e-3 vs the 2e-2 gate. The scan runs
in fp32; only the gated scan output (the moving operand of the output GEMM)
and the y partials written back to HBM are bf16.

DMA: aggregate bandwidth measures ~175GB/s no matter how many queues are
used, so startup is choreographed around strict priority: w0/x0 stream as
interleaved 128KB k-slices (sync/gpsimd queues), then w1,w2,w3 as k-slices,
then x(1), wo. Chunk 0's four projection blocks run K-OUTER across four
live psum banks so the PE consumes each arriving slice with 4 matmuls
instead of waiting for whole tensors, and a burst of dummy matmuls on
zeroed scratch warms the PE DVFS ramp before the first slice lands. y is
written back with one DMA per sequence chunk from a batched [P, MT*NC]
tile; psum->bf16 casts alternate Vector/Scalar so the output-GEMM psum
rotation never waits on a busy Vector engine.

Activation tables: sigmoid is computed as (1 + tanh(z/2))/2 with the 1/2
input scale folded into W_select/W_gate and the output scales into W_out
(W_out/4), so every psum-draining activation (delta-Exp + 12 Tanh) lives in
the `exp_and_others` table set. softplus(z) = ln(exp(z)+1) needs Ln from a
different set: the [4x Ln][4x decay-Exp] block (SBUF-only, 2 table loads)
is tucked into the middle of the Tanh phase where the ACT engine runs far
ahead of the PE's psum-drain needs, so the PE never waits on a table
switch. The ACT order is pinned chunk by chunk.

The last chunk's scan/gate/output GEMM run in two 256-wide halves with the
casts split across Vector and Scalar and two stores per half, so the final
HBM flush shrinks and starts earlier.
"""

import sys

if "/opt/trn_rl_repo" not in sys.path:
    sys.path.insert(0, "/opt/trn_rl_repo")

import numpy as np

# Problem constants (hardcoded per harness contract).
B, S, D = 4, 4096, 1024
P = 128           # SBUF partitions
E = D // 2        # per-core channel slice
NC = 512          # sequence chunk (= one PSUM bank of fp32)
KT = D // P       # k-tiles per input projection contraction (8)
ET = E // P       # e-tiles per core (4)
KO = E // P       # k'-tiles for the output GEMM contraction (4)
MT = D // P       # output-row tiles (8)
SC = S // NC      # sequence chunks (8)
HC = NC // 2      # half-chunk for the tail drain (256)
N_WARM = 11       # dummy warmup matmuls (PE DVFS ramp during startup DMAs)
N_CORES = 8

_CACHE = {}


def _build_program():
    import concourse.bacc as bacc
    import concourse.mybir as mybir
    import concourse.tile as tile
    from concourse.tile import add_dep_helper
    from contextlib import ExitStack

    f32 = mybir.dt.float32
    bf16 = mybir.dt.bfloat16
    AF = mybir.ActivationFunctionType
    OP = mybir.AluOpType

    nc = bacc.Bacc("TRN2", target_bir_lowering=False)

    # Host pre-arranges everything partition-major so each partition's slice
    # of any single DMA is contiguous DRAM.
    xt = nc.declare_dram_parameter("xt", [P, SC, KT, NC], bf16, isOutput=False)
    w4 = nc.declare_dram_parameter("w4", [4, P, KT, E], bf16, isOutput=False)
    wo = nc.declare_dram_parameter("wo", [P, KO * D], bf16, isOutput=False)
    na = nc.declare_dram_parameter("na", [P, ET], f32, isOutput=False)
    yt = nc.declare_dram_parameter("yt", [P, SC, MT, NC], bf16, isOutput=True)

    with tile.TileContext(nc) as tc, ExitStack() as ctx:
        wpool = ctx.enter_context(tc.tile_pool(name="w", bufs=1))
        xpool = ctx.enter_context(tc.tile_pool(name="x", bufs=2))
        ppd = ctx.enter_context(tc.tile_pool(name="ppd", bufs=3, space="PSUM"))
        pp = ctx.enter_context(tc.tile_pool(name="pp", bufs=3, space="PSUM"))
        pyp = ctx.enter_context(tc.tile_pool(name="pyp", bufs=2, space="PSUM"))
        dpool = ctx.enter_context(tc.tile_pool(name="dp", bufs=2))
        decpool = ctx.enter_context(tc.tile_pool(name="dec", bufs=8))
        spool = ctx.enter_context(tc.tile_pool(name="sp", bufs=4))
        tpool = ctx.enter_context(tc.tile_pool(name="tp", bufs=4))
        upool = ctx.enter_context(tc.tile_pool(name="up", bufs=3))
        stpool = ctx.enter_context(tc.tile_pool(name="stp", bufs=6))
        gpool = ctx.enter_context(tc.tile_pool(name="gp", bufs=4))
        gdpool = ctx.enter_context(tc.tile_pool(name="gdp", bufs=8))
        ypool = ctx.enter_context(tc.tile_pool(name="yp", bufs=2))

        # Pin the ACT instruction order to the emission order so the
        # scheduler cannot move a psum-draining activation behind a
        # table-switching phase from another chunk.
        last_act = [None]

        def act(*args, **kwargs):
            h = nc.scalar.activation(*args, **kwargs)
            if last_act[0] is not None:
                add_dep_helper(h.ins, last_act[0].ins, sync=False,
                               reason="pin ACT table phase order")
            last_act[0] = h
            return h

        # PE warmup scratch.
        xz = wpool.tile([P, NC], bf16, name="xz", tag="xz")
        wz = wpool.tile([P, P], bf16, name="wz", tag="wz")
        nc.vector.memset(xz[:, :], 0.0)
        nc.vector.memset(wz[:, :], 0.0)

        # Startup DMA choreography (shared ~175GB/s pipe, strict priority):
        #   gpsimd: na, x0 k-slices          (x0 paired with w0 stream)
        #   sync:   w0 k-slices, w1/w2/w3 k-slices, x1 k-slices, wo
        w_sb = [None] * 4
        for q in range(4):
            w_sb[q] = wpool.tile([P, KT * E], bf16, name=f"w{q}_sb", tag=f"w{q}")
        x_next = xpool.tile([P, KT * NC], bf16, name="x_sb", tag="x")
        na_sb = wpool.tile([P, ET], f32, name="na_sb", tag="na")
        wo_sb = wpool.tile([P, KO * D], bf16, name="wo_sb", tag="wo")

        for k in range(KT):
            nc.sync.dma_start(w_sb[0][:, k * E:(k + 1) * E], w4[0, :, k])
            nc.gpsimd.dma_start(
                x_next[:, k * NC:(k + 1) * NC], xt[:, 0, k])
        nc.gpsimd.dma_start(na_sb[:, :], na[:, :])
        for q in (1, 2, 3):
            for k in range(KT):
                nc.sync.dma_start(w_sb[q][:, k * E:(k + 1) * E], w4[q, :, k])
        x1 = xpool.tile([P, KT * NC], bf16, name="x_sb", tag="x")
        nc.sync.dma_start(x1.rearrange("p (k s) -> p k s", k=KT), xt[:, 1])
        nc.sync.dma_start(wo_sb[:, :], wo[:, :])

        def load_x(c):
            # Two half-DMAs instead of one 1MB burst: smoother SBUF-write
            # and DMA-pipe interference with the compute stream.
            x_sb = xpool.tile([P, KT * NC], bf16, name="x_sb", tag="x")
            KH = KT // 2
            for half in range(2):
                nc.gpsimd.dma_start(
                    x_sb[:, half * KH * NC:(half + 1) * KH * NC].rearrange(
                        "p (k s) -> p k s", k=KH),
                    xt[:, c, half * KH:(half + 1) * KH])
            return x_sb

        # Dummy warmup matmuls (zeros in, never read back).
        warm_ps = ppd.tile([P, NC], f32, name="warm", tag="ppd")
        for _ in range(N_WARM):
            nc.tensor.matmul(warm_ps[:, :], wz[:, :], xz[:, :],
                             start=True, stop=True)

        prev_states = [None] * ET
        gated_tiles = [[None] * ET for _ in range(SC)]
        front_state = {}

        def mm_group(ps, q, j, x_sb):
            for k in range(KT):
                nc.tensor.matmul(
                    ps[:, :],
                    w_sb[q][:, k * E + j * P: k * E + (j + 1) * P],
                    x_sb[:, k * NC:(k + 1) * NC],
                    start=(k == 0),
                    stop=(k == KT - 1),
                )

        def mm_block_kouter(q, banks, x_sb):
            ps = [pool.tile([P, NC], f32, name=f"ps0_{q}{j}", tag=tag)
                  for j, (pool, tag) in enumerate(banks)]
            for k in range(KT):
                for j in range(ET):
                    nc.tensor.matmul(
                        ps[j][:, :],
                        w_sb[q][:, k * E + j * P: k * E + (j + 1) * P],
                        x_sb[:, k * NC:(k + 1) * NC],
                        start=(k == 0),
                        stop=(k == KT - 1),
                    )
            return ps

        def lphase(e1_t):
            # SBUF-only softplus tail + decay (2 table loads live here):
            # ln(e1+1), then decay = exp(-a * softplus).
            l1_t = []
            for j in range(ET):
                l1 = dpool.tile([P, NC], f32, name="l1", tag="l1", bufs=5)
                act(l1[:, :], e1_t[j][:, :], AF.Ln, bias=1.0)
                l1_t.append(l1)
            dec_t = []
            for j in range(ET):
                d = decpool.tile([P, NC], f32, name="dec", tag="dec")
                act(d[:, :], l1_t[j][:, :], AF.Exp, scale=na_sb[:, j:j + 1])
                dec_t.append(d)
            return dec_t

        BQ = ((1, spool, "tS"), (2, tpool, "tI"), (3, gpool, "tG"))

        def emit_front0(x_sb):
            # Chunk 0: K-OUTER blocks over 4 live psum banks per projection
            # so the PE consumes every k-slice the moment it lands.
            banksA = [(ppd, "ppd"), (ppd, "ppd"), (ppd, "ppd"), (pp, "pp")]
            banksB = [(pp, "pp"), (pp, "pp"), (pyp, "py"), (pyp, "py")]
            psA = mm_block_kouter(0, banksA, x_sb)
            e1_t = []
            for j in range(ET):
                e1 = dpool.tile([P, NC], f32, name="e1", tag="e1", bufs=5)
                act(e1[:, :], psA[j][:, :], AF.Exp)
                e1_t.append(e1)
            outs = []
            dec_t = None
            for qi, (q, pool, nm) in enumerate(BQ):
                psB = mm_block_kouter(q, banksB if qi % 2 == 0 else banksA,
                                      x_sb)
                ts = []
                for j in range(ET):
                    t = pool.tile([P, NC], f32, name=nm, tag=nm)
                    act(t[:, :], psB[j][:, :], AF.Tanh)
                    ts.append(t)
                outs.append(ts)
                if qi == 0:
                    dec_t = lphase(e1_t)
            front_state[0] = (dec_t, outs[0], outs[1], outs[2])

        def emit_front(c, x_sb):
            # Phase A: delta projection; Exp drains each psum as it fills.
            e1_t = []
            for j in range(ET):
                ps = ppd.tile([P, NC], f32, name="psd", tag="ppd")
                mm_group(ps, 0, j, x_sb)
                e1 = dpool.tile([P, NC], f32, name="e1", tag="e1", bufs=5)
                act(e1[:, :], ps[:, :], AF.Exp)
                e1_t.append(e1)
            # SBUF-only Ln/decay block right after the delta drains (its 2
            # table loads run while the PE is still in the delta block's
            # shadow and the Tanh drains haven't started).
            dec_t = lphase(e1_t)
            # Phase B: the three gate projections, q-outer, cycling psum
            # banks across BOTH the pp and ppd pools (6-deep rotation, so
            # a B-group never waits on a Tanh drain only 3 groups back).
            outs = [[], [], []]
            nb = 0
            for qi, (q, pool, nm) in enumerate(BQ):
                for j in range(ET):
                    bp, btag = ((pp, "pp") if (nb // 3) % 2 == 0
                                else (ppd, "ppd"))
                    ps = bp.tile([P, NC], f32, name="ps", tag=btag)
                    mm_group(ps, q, j, x_sb)
                    t = pool.tile([P, NC], f32, name=nm, tag=nm)
                    act(t[:, :], ps[:, :], AF.Tanh)
                    outs[qi].append(t)
                    nb += 1
            front_state[c] = (dec_t, outs[0], outs[1], outs[2])

        def phase_c(c):
            # update' = (1+tS)*tI ; scan ; gated' = (1+tG)*st  (bf16 out).
            dec_t, tS, tI, tG = front_state.pop(c)
            for j in range(ET):
                u = upool.tile([P, NC], f32, name="upd", tag="upd")
                nc.vector.scalar_tensor_tensor(
                    u[:, :], tS[j][:, :], 1.0, tI[j][:, :],
                    op0=OP.add, op1=OP.mult,
                )
                st = stpool.tile([P, NC], f32, name="st", tag="st")
                init = 0.0 if c == 0 else prev_states[j][:, NC - 1:NC]
                nc.vector.tensor_tensor_scan(
                    st[:, :], dec_t[j][:, :], u[:, :], init,
                    op0=OP.mult, op1=OP.add,
                )
                prev_states[j] = st
                g = gdpool.tile([P, NC], bf16, name="gated", tag="gated")
                nc.vector.scalar_tensor_tensor(
                    g[:, :], tG[j][:, :], 1.0, st[:, :],
                    op0=OP.add, op1=OP.mult,
                )
                gated_tiles[c][j] = g

        def emit_back(c):
            # Output GEMM for chunk c; psum->bf16 casts alternate between
            # Vector and Scalar so the pyp rotation never waits on a busy
            # Vector engine; one contiguous DMA per chunk.
            y_big = ypool.tile([P, MT * NC], bf16, name="y_big", tag="ybig")
            for m in range(MT):
                py = pyp.tile([P, NC], f32, name="py", tag="py")
                for j in range(KO):
                    nc.tensor.matmul(
                        py[:, :],
                        wo_sb[:, j * D + m * P: j * D + (m + 1) * P],
                        gated_tiles[c][j][:, :],
                        start=(j == 0),
                        stop=(j == KO - 1),
                    )
                dst = y_big[:, m * NC:(m + 1) * NC]
                if m % 2 == 0:
                    nc.vector.tensor_copy(dst, py[:, :])
                else:
                    nc.scalar.activation(dst, py[:, :], AF.Copy)
                if m == MT // 2 - 1:
                    nc.sync.dma_start(
                        yt[:, c, 0:MT // 2],
                        y_big[:, :MT // 2 * NC].rearrange(
                            "p (m s) -> p m s", m=MT // 2))
            nc.sync.dma_start(
                yt[:, c, MT // 2:],
                y_big[:, MT // 2 * NC:].rearrange(
                    "p (m s) -> p m s", m=MT // 2))
            gated_tiles[c] = [None] * KO

        def emit_back_final(c):
            # Tail: last chunk's scan/gate/output GEMM in two 256-wide
            # halves; all 8 PSUM banks are free, accumulate j-major into 8
            # live half-bank psums; casts alternate Vector/Scalar; two
            # stores per half so the final flush overlaps the casts.
            dec_t, tS, tI, tG = front_state.pop(c)
            pools = [
                (pyp, "py"), (pyp, "py"), (pp, "pp"), (pp, "pp"),
                (pp, "pp"), (ppd, "ppd"), (ppd, "ppd"), (ppd, "ppd"),
            ]
            st_prev = prev_states
            for half in range(2):
                hsl = slice(half * HC, (half + 1) * HC)
                g_h = []
                for j in range(ET):
                    u = upool.tile([P, HC], f32, name="updh", tag="upd")
                    nc.vector.scalar_tensor_tensor(
                        u[:, :], tS[j][:, hsl], 1.0, tI[j][:, hsl],
                        op0=OP.add, op1=OP.mult,
                    )
                    st = stpool.tile([P, HC], f32, name="sth", tag="st")
                    init = st_prev[j][:, -1:]
                    nc.vector.tensor_tensor_scan(
                        st[:, :], dec_t[j][:, hsl], u[:, :], init,
                        op0=OP.mult, op1=OP.add,
                    )
                    st_prev[j] = st
                    g = gdpool.tile([P, HC], bf16, name="gatedh", tag="gated")
                    nc.vector.scalar_tensor_tensor(
                        g[:, :], tG[j][:, hsl], 1.0, st[:, :],
                        op0=OP.add, op1=OP.mult,
                    )
                    g_h.append(g)
                y_ps = [
                    pool.tile([P, HC], f32, name=f"pyf{m}", tag=tag)
                    for m, (pool, tag) in enumerate(pools)
                ]
                y_big = ypool.tile([P, MT * HC], bf16, name="y_bigh",
                                   tag="ybig")
                for j in range(KO - 1):
                    for m in range(MT):
                        nc.tensor.matmul(
                            y_ps[m][:, :],
                            wo_sb[:, j * D + m * P: j * D + (m + 1) * P],
                            g_h[j][:, :],
                            start=(j == 0),
                            stop=False,
                        )
                j = KO - 1
                for m in range(MT):
                    nc.tensor.matmul(
                        y_ps[m][:, :],
                        wo_sb[:, j * D + m * P: j * D + (m + 1) * P],
                        g_h[j][:, :],
                        start=False,
                        stop=True,
                    )
                    dst = y_big[:, m * HC:(m + 1) * HC]
                    if m % 2 == 0:
                        nc.vector.tensor_copy(dst, y_ps[m][:, :])
                    else:
                        nc.scalar.activation(dst, y_ps[m][:, :], AF.Copy)
                    if m == 3:
                        nc.sync.dma_start(
                            yt[:, c, 0:4, hsl],
                            y_big[:, 0:4 * HC].rearrange(
                                "p (m s) -> p m s", m=4))
                nc.sync.dma_start(
                    yt[:, c, 4:8, hsl],
                    y_big[:, 4 * HC:].rearrange("p (m s) -> p m s", m=4))

        for c in range(SC):
            x_cur = x_next
            if c == 0:
                x_next = x1
            elif c + 1 < SC:
                x_next = load_x(c + 1)
            if c == 0:
                emit_front0(x_cur)
            else:
                emit_front(c, x_cur)
            if c < SC - 1:
                phase_c(c)
            if c > 0:
                emit_back(c - 1)
        emit_back_final(SC - 1)

    nc.compile()
    return nc


def _get_program():
    if "nc" not in _CACHE:
        _CACHE["nc"] = _build_program()
    return _CACHE["nc"]


def _make_in_maps(x, W_in, W_select, W_gate, W_out, W_delta, log_a):
    import ml_dtypes

    bf = ml_dtypes.bfloat16
    a = (1.0 / (1.0 + np.exp(-log_a.astype(np.float32)))).astype(np.float32)
    in_maps = []
    for c in range(N_CORES):
        b, h = divmod(c, 2)
        sl = slice(h * E, (h + 1) * E)
        # xt[p, c, k, n] = x[b, c*NC+n, k*P+p]
        xT = np.ascontiguousarray(
            x[b].T.astype(bf).reshape(KT, P, SC, NC).transpose(1, 2, 0, 3)
        )                                                       # [P,SC,KT,NC]
        # w4[q, p, k, e] = Wq_eff.T[k*P+p, e]
        w4 = np.ascontiguousarray(
            np.stack(
                [
                    W_delta[sl, :].T,
                    0.5 * W_select[sl, :].T,   # sigmoid via tanh(z/2)
                    W_in[sl, :].T,
                    0.5 * W_gate[sl, :].T,     # sigmoid via tanh(z/2)
                ]
            ).astype(bf).reshape(4, KT, P, E).transpose(0, 2, 1, 3)
        )
        # wo[p, j*D+d] = Wo_eff.T[j*P+p, d]
        wo = np.ascontiguousarray(
            (0.25 * W_out[:, sl].T).astype(bf)
            .reshape(KO, P, D).transpose(1, 0, 2).reshape(P, KO * D)
        )
        na_m = np.ascontiguousarray((-a[sl]).reshape(ET, P).T)  # [P, ET]
        in_maps.append({"xt": xT, "w4": w4, "wo": wo, "na": na_m})
    return in_maps


def _gather(results):
    y = np.empty((B, S, D), np.float32)
    for b in range(B):
        yT = np.zeros((D, S), np.float32)
        for r in (results[2 * b], results[2 * b + 1]):
            # yt[p, c, m, n] = y_T[m*P+p, c*NC+n]
            arr = r["yt"].astype(np.float32)
            yT += arr.transpose(2, 0, 1, 3).reshape(D, S)
        y[b] = yT.T
    return y


def kernel(x, W_in, W_select, W_gate, W_out, W_delta, log_a):
    from concourse.bass_utils import run_bass_kernel_spmd

    nc = _get_program()
    in_maps = _make_in_maps(
        np.asarray(x, np.float32),
        np.asarray(W_in, np.float32),
        np.asarray(W_select, np.float32),
        np.asarray(W_gate, np.float32),
        np.asarray(W_out, np.float32),
        np.asarray(W_delta, np.float32),
        np.asarray(log_a, np.float32),
    )
    res = run_bass_kernel_spmd(nc, in_maps, core_ids=list(range(N_CORES)))
    return _gather(res.results)


if __name__ == "__main__":
    nc = _get_program()
    print("program built OK")
